# revision 1
# baseline (speedup 1.0000x reference)
"""Trainium2 Bass kernel for XCiT-style channel ("cross-covariance") attention.

Reference computation (per batch element b):
    qkv  = x @ w_qkv.T                    # [N, 3C]
    q,k,v -> [H, DH, N] (channel-major)
    q,k  l2-normalized along N (tokens)
    attn = softmax((q @ k^T) * temp)      # [H, DH, DH]
    out  = (attn @ v) -> [N, C] @ w_proj.T

Shapes: B=8, N=4096, C=512, H=8, DH=64.

Strategy: data-parallel over batch across the 8 NeuronCores (one batch
element per core, weights replicated, no collectives).

Key algebraic restructure: v is never materialized. Since
    out = (attn @ v)^T @ w_proj^T  and  v = w_v @ x^T,
fold attention into the projection:
    weff[he, co] = sum_d attn_h[d, e] * w_proj^T[hd, co]
    G = w_v^T @ weff          # [cin, cout], per batch element
    out = x @ G
This replaces the v-projection GEMM and the attn@v GEMM with one small
G GEMM plus a single x@G pass.

Precision: q,k path runs in fp8-e4m3 with DoubleRow matmuls (2x PE rate,
softmax normalization washes out the quantization error); the output
path (x@G) stays bf16. Norms: ||q|| comes free from the diagonal of a
fused q@[q|k] gram (q and k are stored interleaved per head-pair to
make the fused gram contiguous); ||k|| via a ones^T @ k^2 DoubleRow
matmul. rsqrt is computed as exp(-0.5*ln(s)) so the ACT engine needs
only the natural_log_exp function table (no per-iteration table swaps).

All inputs are pre-transposed / pre-cast on the host so every DMA is a
plain contiguous load (no XBAR-transposed descriptors).
"""

import numpy as np

import concourse.bacc as bacc
import concourse.mybir as mybir
import concourse.tile as tile

F32 = mybir.dt.float32
BF16 = mybir.dt.bfloat16
F8 = mybir.dt.float8e4
DR = mybir.MatmulPerfMode.DoubleRow

N_TOK = 4096
C = 512
H = 8
DH = 64
P = 128
KT = C // P            # 4 cin tiles
NT = N_TOK // P        # 32 token tiles
NTP = NT // 2          # 16 token tile pairs
NCH = N_TOK // 512     # 8 output chunks
TPC = 4                # token tiles per chunk
N_CORES = 8

CFG = {"phases": "WABC", "psqk_bufs": 3, "psy_bufs": 3, "hint": True, "gram_pipe": True}


def build_bass(loop_n=None):
    nc = bacc.Bacc()

    xT8_d = nc.declare_dram_parameter("xT8", [C, N_TOK], F8, isOutput=False)
    xTb_d = nc.declare_dram_parameter("xTb", [C, N_TOK], BF16, isOutput=False)
    wqk_d = nc.declare_dram_parameter("wqkI8", [C, 2 * C], F8, isOutput=False)
    wv_d = nc.declare_dram_parameter("wv", [C, C], BF16, isOutput=False)
    wpT_d = nc.declare_dram_parameter("wpT", [C, C], BF16, isOutput=False)
    temp_d = nc.declare_dram_parameter("temperature", [H, 1, 1], F32, isOutput=False)
    id_d = nc.declare_dram_parameter("ident", [P, P], F32, isOutput=False)
    out_d = nc.declare_dram_parameter("out", [N_TOK, C], BF16, isOutput=True)

    with tile.TileContext(nc) as tc:
        with tc.tile_pool(name="persist", bufs=1) as persist:
            # [P, 2, 16] so the DoubleRow Ldweights outer free step is
            # 16B-aligned (dual-fp8 ISA restriction); only [:, :, 0:1] is used
            ones8 = persist.tile([P, 2, 16], F8, tag="ones8")
            nc.gpsimd.memset(ones8[:], 1.0)
            idm = persist.tile([P, P], F32, tag="idm")
            nc.sync.dma_start(out=idm[:], in_=id_d[:])
            # t8j[0, two*4 + j] = temperature[2j + two]  (two-major order)
            t8 = persist.tile([1, H], F32, tag="t8")
            for two in range(2):
                nc.sync.dma_start(
                    out=t8[0:1, two * KT:(two + 1) * KT],
                    in_=temp_d.rearrange("(j two) a b -> (a b) two j", two=2)
                    [:, two, :],
                )
            tcol = persist.tile([P, KT], F32, tag="tcol")
            # tcol[p, j] = temperature[2j + p//64]
            for two in range(2):
                nc.sync.dma_start(
                    out=tcol[two * DH:(two + 1) * DH, :],
                    in_=t8[0:1, two * KT:(two + 1) * KT]
                    .unsqueeze(1).broadcast_to((1, DH, KT)),
                )

            # persistent SBUF tensors
            xT8 = persist.tile([P, KT, N_TOK], F8, tag="xT8")
            xTb = persist.tile([P, KT, N_TOK], BF16, tag="xTb")
            wqk = persist.tile([P, KT, 2 * C], F8, tag="wqk")
            wv_sb = persist.tile([P, KT, C], BF16, tag="wv_sb")
            wpT_sb = persist.tile([P, KT, C], BF16, tag="wpT_sb")
            qk_sb = persist.tile([P, NT, 2 * C], F8, tag="qk_sb")
            weff = persist.tile([P, KT, C], BF16, tag="weff")
            g_sb = persist.tile([P, KT, C], BF16, tag="g_sb")
            d2 = persist.tile([P, KT], F32, tag="d2")
            rq_col = persist.tile([P, KT], F32, tag="rq_col")
            rk_row = persist.tile([1, C], F32, tag="rk_row")
            rk_bcast = persist.tile([P, C], F32, tag="rk_bcast")

            locals_d = dict(
                ones8=ones8, idm=idm, tcol=tcol, xT8=xT8, xTb=xTb,
                wqk=wqk, wv_sb=wv_sb, wpT_sb=wpT_sb, qk_sb=qk_sb,
                weff=weff, g_sb=g_sb, d2=d2, rq_col=rq_col,
                rk_row=rk_row, rk_bcast=rk_bcast,
                xT8_d=xT8_d, xTb_d=xTb_d, wqk_d=wqk_d, wv_d=wv_d,
                wpT_d=wpT_d, out_d=out_d,
            )

            def phases():
                _emit(nc, tc, locals_d)

            if loop_n is None:
                phases()
            else:
                hint = tuple(nc.engines.keys()) if CFG.get("hint") else ()
                with tc.For_i(0, loop_n, 1, hint_engines=hint):
                    phases()

    nc.compile()
    return nc


def _emit(nc, tc, L):
    ones8, idm, tcol = L["ones8"], L["idm"], L["tcol"]
    xT8, xTb, wqk = L["xT8"], L["xTb"], L["wqk"]
    wv_sb, wpT_sb, qk_sb = L["wv_sb"], L["wpT_sb"], L["qk_sb"]
    weff, g_sb, d2 = L["weff"], L["g_sb"], L["d2"]
    rq_col, rk_row, rk_bcast = L["rq_col"], L["rk_row"], L["rk_bcast"]
    xT8_d, xTb_d, wqk_d = L["xT8_d"], L["xTb_d"], L["wqk_d"]
    wv_d, wpT_d, out_d = L["wv_d"], L["wpT_d"], L["out_d"]
    phases_on = CFG.get("phases", "WABC")

    copy_flip = [0]

    def copy_out(dst_ap, src_ap):
        """PSUM->SBUF evacuation, alternating DVE / ACT."""
        if copy_flip[0] % 2 == 0:
            nc.vector.tensor_copy(out=dst_ap, in_=src_ap)
        else:
            nc.scalar.copy(out=dst_ap, in_=src_ap)
        copy_flip[0] += 1

    if "W" in phases_on:
        # q,k-path inputs first so the PE can start immediately
        for k in range(KT):
            nc.sync.dma_start(out=xT8[:, k, :], in_=xT8_d[k * P:(k + 1) * P, :])
        for k in range(KT):
            nc.sync.dma_start(out=wqk[:, k, :], in_=wqk_d[k * P:(k + 1) * P, :])
        for k in range(KT):
            nc.sync.dma_start(out=wpT_sb[:, k, :], in_=wpT_d[k * P:(k + 1) * P, :])
            nc.sync.dma_start(out=wv_sb[:, k, :], in_=wv_d[k * P:(k + 1) * P, :])
            nc.sync.dma_start(out=xTb[:, k, :], in_=xTb_d[k * P:(k + 1) * P, :])

    psg = tc.alloc_tile_pool(name="psg", bufs=1, space="PSUM")
    gqq = [psg.tile([P, 2 * P], F32, tag=f"gqq{p}", name=f"gqq{p}")
           for p in range(KT)]
    knps = psg.tile([1, C], F32, tag="knps")

    def gq(p, lo, hi):
        """columns [lo:hi) of pair-block p's [128,256] gram"""
        return gqq[p][:, lo:hi]

    if "A" in phases_on:
        with (
            tc.tile_pool(name="psqk", bufs=CFG["psqk_bufs"], space="PSUM") as psqk,
            tc.tile_pool(name="k2p", bufs=3) as k2p,
        ):
            def emit_gram(tp, k2):
                # fused gram: q_p @ [q_p | k_p]  (DoubleRow over the tile pair)
                for p in range(KT):
                    nc.tensor.matmul(
                        gq(p, 0, 2 * P),
                        qk_sb[:, 2 * tp:2 * tp + 2, 2 * P * p:2 * P * p + P],
                        qk_sb[:, 2 * tp:2 * tp + 2, 2 * P * p:2 * P * (p + 1)],
                        start=(tp == 0), stop=(tp == NTP - 1),
                        perf_mode=DR,
                    )
                # ||k||^2: ones^T @ k^2
                nc.tensor.matmul(
                    knps[:], ones8[:, :, 0:1], k2[:],
                    start=(tp == 0), stop=(tp == NTP - 1),
                    perf_mode=DR,
                )

            pending = None  # (tp, k2) whose gram/knorm matmuls are deferred
            for tp in range(NTP):
                # q,k production (fp8 DoubleRow), interleaved [q_p|k_p] layout
                for ti in range(2):
                    t = 2 * tp + ti
                    for half in range(2):
                        ps = psqk.tile([P, C], F32, tag="psqk")
                        for kk in range(2):
                            nc.tensor.matmul(
                                ps[:],
                                xT8[:, 2 * kk:2 * kk + 2, t * P:(t + 1) * P],
                                wqk[:, 2 * kk:2 * kk + 2, half * C:(half + 1) * C],
                                start=(kk == 0), stop=(kk == 1),
                                perf_mode=DR,
                            )
                        copy_out(qk_sb[:, t, half * C:(half + 1) * C], ps[:])
                # previous pair's gram/knorm: their inputs (copies/squares)
                # complete while this pair's qk matmuls stream -> no PE stall
                if pending is not None:
                    emit_gram(*pending)
                    pending = None
                # k^2 (fp8, ACT square) for the pair's two tiles
                k2 = k2p.tile([P, 2, C], F8, tag="k2", name="k2")
                for ti in range(2):
                    t = 2 * tp + ti
                    ksrc = qk_sb[:, t, :].rearrange(
                        "p (b c) -> p b c", c=2 * P
                    )[:, :, P:2 * P]
                    if CFG.get("sq_dve", True):
                        nc.vector.tensor_mul(
                            out=k2[:, ti, :].rearrange("p (b c) -> p b c", c=P),
                            in0=ksrc, in1=ksrc,
                        )
                    else:
                        nc.scalar.square(
                            out=k2[:, ti, :].rearrange("p (b c) -> p b c", c=P),
                            in_=ksrc,
                        )
                if CFG.get("gram_pipe", True):
                    pending = (tp, k2)
                else:
                    emit_gram(tp, k2)
            if pending is not None:
                emit_gram(*pending)

    b_mode = CFG.get("b_mode", "full")
    if "B" in phases_on and b_mode != "none":
        with (
            tc.tile_pool(name="smp", bufs=2) as smp,
            tc.tile_pool(name="psw", bufs=2, space="PSUM") as psw,
        ):
            # rq = temp * rsqrt(diag(qq));  rk = rsqrt(knorm2)
            for p in range(KT):
                scr = smp.tile([P, P], F32, tag="scr")
                nc.vector.tensor_mul(out=scr[:], in0=gq(p, 0, P), in1=idm[:])
                nc.vector.reduce_sum(d2[:, p:p + 1], scr[:],
                                     axis=mybir.AxisListType.X)
            lnq = smp.tile([P, KT], F32, tag="lnq")
            nc.scalar.activation(lnq[:], d2[:], mybir.ActivationFunctionType.Ln)
            rsq = smp.tile([P, KT], F32, tag="rsq")
            nc.scalar.activation(rsq[:], lnq[:], mybir.ActivationFunctionType.Exp,
                                 bias=0.0, scale=-0.5)
            nc.vector.tensor_mul(out=rq_col[:], in0=rsq[:], in1=tcol[:])
            lnk = smp.tile([1, C], F32, tag="lnk")
            nc.scalar.activation(lnk[:], knps[:], mybir.ActivationFunctionType.Ln)
            nc.scalar.activation(rk_row[:], lnk[:],
                                 mybir.ActivationFunctionType.Exp,
                                 bias=0.0, scale=-0.5)
            nc.sync.dma_start(
                out=rk_bcast[:],
                in_=rk_row[0:1, :].unsqueeze(1).broadcast_to((1, P, C)),
            )

            # softmax per head pair -> block-diag attn (bf16) -> weff
            for p in range(KT if b_mode != "norms" else 0):
                abd = smp.tile([P, P], BF16, tag="abd")
                nc.gpsimd.memset(abd[:], 0.0)
                smt = smp.tile([P, P], F32, tag="smt")
                nc.vector.tensor_mul(
                    out=smt[:], in0=gq(p, P, 2 * P),
                    in1=rk_bcast[:, p * P:(p + 1) * P],
                )
                et = smp.tile([P, P], F32, tag="et")
                ssum = smp.tile([P, 1], F32, tag="ssum")
                rs = smp.tile([P, 1], F32, tag="rs")
                for hh in range(2):
                    sl = slice(hh * DH, (hh + 1) * DH)
                    nc.scalar.activation(
                        et[sl, sl], smt[sl, sl],
                        mybir.ActivationFunctionType.Exp,
                        bias=0.0, scale=rq_col[sl, p:p + 1],
                        accum_out=ssum[sl, 0:1],
                    )
                nc.vector.reciprocal(rs[:], ssum[:])
                for hh in range(2):
                    sl = slice(hh * DH, (hh + 1) * DH)
                    nc.vector.tensor_scalar_mul(abd[sl, sl], et[sl, sl],
                                                rs[sl, 0:1])
                wps = psw.tile([P, C], F32, tag="wps")
                nc.tensor.matmul(wps[:], abd[:], wpT_sb[:, p, :],
                                 start=True, stop=True)
                copy_out(weff[:, p, :], wps[:])

            # G = wv^T @ weff
            for j in range(KT if b_mode == "full" else 0):
                gp = psw.tile([P, C], F32, tag="wps", name=f"gp{j}")
                for t in range(KT):
                    nc.tensor.matmul(
                        gp[:], wv_sb[:, t, j * P:(j + 1) * P], weff[:, t, :],
                        start=(t == 0), stop=(t == KT - 1),
                    )
                copy_out(g_sb[:, j, :], gp[:])

    stub = []
    if "B" not in phases_on or b_mode == "none":
        stub = [weff, g_sb, rq_col, rk_bcast, d2, rk_row]
    elif b_mode == "norms":
        stub = [weff, g_sb]
    elif b_mode == "softmax":
        stub = [g_sb]
    for t_ in stub:
        nc.gpsimd.memset(t_[:], 0.0)

    psg.release()

    if "C" in phases_on:
        with (
            tc.tile_pool(name="yp", bufs=3) as yp,
            tc.tile_pool(name="psy", bufs=CFG["psy_bufs"], space="PSUM") as psy,
        ):
            for ch in range(NCH):
                yc = yp.tile([P, TPC, C], BF16, tag="yc")
                for t in range(TPC):
                    g = ch * TPC + t
                    ps = psy.tile([P, C], F32, tag="psy")
                    for k in range(KT):
                        nc.tensor.matmul(
                            ps[:], xTb[:, k, g * P:(g + 1) * P], g_sb[:, k, :],
                            start=(k == 0), stop=(k == KT - 1),
                        )
                    copy_out(yc[:, t, :], ps[:])
                nc.sync.dma_start(
                    out=out_d[ch * C:(ch + 1) * C, :].rearrange(
                        "(t p) c -> p t c", p=P
                    ),
                    in_=yc[:],
                )
    else:
        for ch in range(NCH):
            nc.sync.dma_start(
                out=out_d[ch * C:(ch + 1) * C, :].rearrange(
                    "(t p) c -> p t c", p=P
                ),
                in_=g_sb[:],
            )


_NC_CACHE = None


def _get_nc():
    global _NC_CACHE
    if _NC_CACHE is None:
        _NC_CACHE = build_bass()
    return _NC_CACHE


def make_in_maps(x, w_qkv, w_proj, temperature):
    """Host-side prep: transpose/cast/interleave so the kernel only does
    plain contiguous DMA loads."""
    import ml_dtypes

    bf = ml_dtypes.bfloat16
    f8 = ml_dtypes.float8_e4m3
    x = np.asarray(x, dtype=np.float32)
    w_qkv = np.asarray(w_qkv, dtype=np.float32)
    w_proj = np.asarray(w_proj, dtype=np.float32)
    temperature = np.ascontiguousarray(np.asarray(temperature, dtype=np.float32))

    # interleaved q/k weight blocks: [q_p0 k_p0 q_p1 k_p1 ...] as columns of
    # the transposed weight [cin, 1024]
    wq = w_qkv[0:C]          # [512 qch, 512 cin]
    wk = w_qkv[C:2 * C]
    blocks = []
    for j in range(KT):
        blocks.append(wq[j * P:(j + 1) * P].T)   # [cin, 128]
        blocks.append(wk[j * P:(j + 1) * P].T)
    wqkI8 = np.ascontiguousarray(np.concatenate(blocks, axis=1).astype(f8))

    wv = np.ascontiguousarray(w_qkv[2 * C:3 * C].astype(bf))      # [he, cin]
    wpT = np.ascontiguousarray(w_proj.T.astype(bf))               # [hd, cout]
    ident = np.eye(P, dtype=np.float32)

    maps = []
    for b in range(N_CORES):
        xT = np.ascontiguousarray(x[b].T)
        maps.append({
            "xT8": xT.astype(f8),
            "xTb": xT.astype(bf),
            "wqkI8": wqkI8,
            "wv": wv,
            "wpT": wpT,
            "temperature": temperature,
            "ident": ident,
        })
    return maps


def kernel(**inputs) -> np.ndarray:
    from concourse.bass_utils import run_bass_kernel_spmd

    nc = _get_nc()
    in_maps = make_in_maps(
        inputs["x"], inputs["w_qkv"], inputs["w_proj"], inputs["temperature"]
    )
    res = run_bass_kernel_spmd(nc, in_maps, core_ids=list(range(N_CORES)))
    return np.stack(
        [np.asarray(res.results[b]["out"], dtype=np.float32)
         for b in range(N_CORES)],
        axis=0,
    )



# revision 3
# speedup vs baseline: 1.5593x; 1.5593x over previous
"""Trainium2 Bass kernel for XCiT-style channel ("cross-covariance") attention.

Reference computation (per batch element b):
    qkv  = x @ w_qkv.T                    # [N, 3C]
    q,k,v -> [H, DH, N] (channel-major)
    q,k  l2-normalized along N (tokens)
    attn = softmax((q @ k^T) * temp)      # [H, DH, DH]
    out  = (attn @ v) -> [N, C] @ w_proj.T

Shapes: B=8, N=4096, C=512, H=8, DH=64.

Strategy: data-parallel over batch across the 8 NeuronCores (one batch
element per core, weights replicated, no collectives).

Key algebraic restructure v2: neither q, k nor v are ever materialized.
Everything the attention needs is a function of the token Gram matrix
    Xg = x^T x                          # [C, C], per batch element
since
    q_h^T k_h   = wq_h Xg wk_h^T        # per-head [DH, DH] logits
    ||q_d||^2   = (wq Xg wq^T)[d, d]
    ||k_e||^2   = (wk Xg wk^T)[e, e]
and (folding v and the projection, as before)
    weff[he, co] = sum_d attn_h[d, e] * w_proj^T[hd, co]
    G = w_v^T @ weff                    # [cin, cout]
    out = x @ G
This removes the [N, 2C] q/k intermediate entirely: the only O(N)
matmuls left are Xg (fp8 DoubleRow, 2x PE rate) and x @ G (bf16).

Pipeline per core:
    Xg   = x8^T x8          fp8 DR, accumulate over token-tile pairs
    Z    = Xg @ [wq^T|wk^T] bf16    ([cin, 2C]; Xg symmetric -> Z = [wq Xg | wk Xg]^T)
    QS_p = Zq_p^T. gram: [Q2_p | S_p] = Z_p^T over cin vs [wq_p | wk_p]
    dk   = ones^T (Zk o wk^T)       row of ||k||^2 via ones-matmul
    softmax (rq = temp*rsqrt(diag Q2), rk = rsqrt(dk)) -> attn block-diag
    weff = attn @ wpT;  G = wv^T weff;  out = x @ G
"""

import numpy as np

import concourse.bacc as bacc
import concourse.mybir as mybir
import concourse.tile as tile

F32 = mybir.dt.float32
BF16 = mybir.dt.bfloat16
F8 = mybir.dt.float8e4
DR = mybir.MatmulPerfMode.DoubleRow

N_TOK = 4096
C = 512
H = 8
DH = 64
P = 128
KT = C // P            # 4 cin tiles
NT = N_TOK // P        # 32 token tiles
NTP = NT // 2          # 16 token tile pairs
NCH = N_TOK // 512     # 8 output chunks
TPC = 4                # token tiles per chunk
N_CORES = 8

CFG = {"phases": "WABC", "psy_bufs": 3, "hint": True, "xg_dr": True}

_COMBINED_TABLE = "natural_log_exp_and_others"


class _Bacc(bacc.Bacc):
    """Bacc with a single combined ACT function table.

    The stock inserter picks the first table containing each activation
    function (Ln -> natural_log, Exp/Copy -> exp_and_others), so a
    Copy/Ln/Exp mix thrashes 1.28us table loads right on the softmax
    critical path. Every function this kernel uses (copy, ln, exp) lives
    in natural_log_exp_and_others, so retarget all loads there and drop
    the now-redundant ones (they carry no sync info).
    """

    def insert_act_table_loads(self):
        super().insert_act_table_loads()
        from concourse.hw_specs import get_activation_tables

        tables = get_activation_tables(self.m.arch)
        names = list(tables)
        combined_id = names.index(_COMBINED_TABLE)
        allowed = tables[_COMBINED_TABLE]
        for b in self.main_func.blocks:
            first = True
            keep = []
            for inst in b.instructions:
                if isinstance(inst, mybir.InstActivation):
                    assert inst.func in allowed, inst.func
                if isinstance(inst, mybir.InstLoadActFuncSet):
                    si = inst.sync_info
                    assert si is None or (not si.on_wait and not si.on_update)
                    if first:
                        inst.act_func_set_id = combined_id
                        first = False
                    else:
                        continue
                keep.append(inst)
            b.instructions[:] = keep


def build_bass(loop_n=None):
    nc = _Bacc() if CFG.get("act_fix", True) else bacc.Bacc()

    x8_d = nc.declare_dram_parameter("x8", [N_TOK, C], F8, isOutput=False)
    xTb_d = nc.declare_dram_parameter("xTb", [C, N_TOK], BF16, isOutput=False)
    wqkT_d = nc.declare_dram_parameter("wqkT", [C, 2 * C], BF16, isOutput=False)
    wv_d = nc.declare_dram_parameter("wv", [C, C], BF16, isOutput=False)
    wpT_d = nc.declare_dram_parameter("wpT", [C, C], BF16, isOutput=False)
    temp_d = nc.declare_dram_parameter("temperature", [H, 1, 1], F32, isOutput=False)
    id_d = nc.declare_dram_parameter("ident", [P, P], F32, isOutput=False)
    out_d = nc.declare_dram_parameter("out", [N_TOK, C], BF16, isOutput=True)

    with tile.TileContext(nc) as tc:
        with tc.tile_pool(name="persist", bufs=1) as persist:
            onesb = persist.tile([P, 1], BF16, tag="onesb")
            nc.gpsimd.memset(onesb[:], 1.0)
            ones2 = persist.tile([P, P], BF16, tag="ones2")
            nc.gpsimd.memset(ones2[:], 1.0)
            # additive block-diagonal mask: 0 on the per-head diagonal
            # blocks, -30 off them (exp -> ~1e-13, vanishes in the softmax)
            mask_bd = persist.tile([P, P], F32, tag="mask_bd")
            nc.gpsimd.memset(mask_bd[:], -3000.0)
            nc.gpsimd.memset(mask_bd[0:DH, 0:DH], 0.0)
            nc.gpsimd.memset(mask_bd[DH:P, DH:P], 0.0)
            idm = persist.tile([P, P], F32, tag="idm")
            scrp = persist.tile([P, P], F32, tag="scrp")
            t8 = persist.tile([1, H], F32, tag="t8")
            tcol = persist.tile([P, KT], F32, tag="tcol")

            # persistent SBUF tensors
            x8 = persist.tile([P, NT, C], F8, tag="x8")
            xTb = persist.tile([P, KT, N_TOK], BF16, tag="xTb")
            wqkT = persist.tile([P, KT, 2 * C], BF16, tag="wqkT")
            wv_sb = persist.tile([P, KT, C], BF16, tag="wv_sb")
            wpT_sb = persist.tile([P, KT, C], BF16, tag="wpT_sb")
            xg_sb = persist.tile([P, KT, C], BF16, tag="xg_sb")
            z_sb = persist.tile([P, KT, 2 * C], BF16, tag="z_sb")
            pk_sb = persist.tile([P, KT, C], BF16, tag="pk_sb")
            weff = persist.tile([P, KT, C], BF16, tag="weff")
            g_sb = persist.tile([P, KT, C], BF16, tag="g_sb")
            d2 = persist.tile([P, KT], F32, tag="d2")
            rq_col = persist.tile([P, KT], F32, tag="rq_col")
            rk_row = persist.tile([1, C], F32, tag="rk_row")
            rk_bcast = persist.tile([P, C], F32, tag="rk_bcast")

            locals_d = dict(
                onesb=onesb, ones2=ones2, mask_bd=mask_bd, idm=idm,
                scrp=scrp, t8=t8,
                tcol=tcol, temp_d=temp_d, id_d=id_d, x8=x8, xTb=xTb,
                wqkT=wqkT, wv_sb=wv_sb, wpT_sb=wpT_sb, xg_sb=xg_sb,
                z_sb=z_sb, pk_sb=pk_sb, weff=weff, g_sb=g_sb, d2=d2,
                rq_col=rq_col, rk_row=rk_row, rk_bcast=rk_bcast,
                x8_d=x8_d, xTb_d=xTb_d, wqkT_d=wqkT_d, wv_d=wv_d,
                wpT_d=wpT_d, out_d=out_d,
            )

            def phases():
                _emit(nc, tc, locals_d)

            if loop_n is None:
                phases()
            else:
                hint = tuple(nc.engines.keys()) if CFG.get("hint") else ()
                with tc.For_i(0, loop_n, 1, hint_engines=hint):
                    phases()

    nc.compile()
    return nc


def _emit(nc, tc, L):
    onesb, ones2, idm, tcol = L["onesb"], L["ones2"], L["idm"], L["tcol"]
    mask_bd, scrp = L["mask_bd"], L["scrp"]
    t8, temp_d, id_d = L["t8"], L["temp_d"], L["id_d"]
    x8, xTb, wqkT = L["x8"], L["xTb"], L["wqkT"]
    wv_sb, wpT_sb = L["wv_sb"], L["wpT_sb"]
    xg_sb, z_sb, pk_sb = L["xg_sb"], L["z_sb"], L["pk_sb"]
    weff, g_sb, d2 = L["weff"], L["g_sb"], L["d2"]
    rq_col, rk_row, rk_bcast = L["rq_col"], L["rk_row"], L["rk_bcast"]
    x8_d, xTb_d, wqkT_d = L["x8_d"], L["xTb_d"], L["wqkT_d"]
    wv_d, wpT_d, out_d = L["wv_d"], L["wpT_d"], L["out_d"]
    phases_on = CFG.get("phases", "WABC")

    copy_flip = [0]

    def copy_out(dst_ap, src_ap):
        """PSUM->SBUF evacuation, alternating DVE / ACT."""
        if copy_flip[0] % 2 == 0:
            nc.vector.tensor_copy(out=dst_ap, in_=src_ap)
        else:
            nc.scalar.copy(out=dst_ap, in_=src_ap)
        copy_flip[0] += 1

    if "W" in phases_on:
        # token-major fp8 x first so Xg can start on the first tile pairs
        x8_src = x8_d.rearrange("(t p) c -> p t c", p=P)
        for lo, hi in ((0, 2), (2, 4), (4, 8), (8, 16)):
            nc.sync.dma_start(out=x8[:, lo:hi, :], in_=x8_src[:, lo:hi, :])
        for k in range(KT):
            nc.sync.dma_start(out=wqkT[:, k, :], in_=wqkT_d[k * P:(k + 1) * P, :])
        for lo, hi in ((16, 24), (24, 32)):
            nc.sync.dma_start(out=x8[:, lo:hi, :], in_=x8_src[:, lo:hi, :])
        # small phase-B constants (needed ~25us in)
        nc.sync.dma_start(out=idm[:], in_=id_d[:])
        # t8[0, two*4 + j] = temperature[2j + two]  (two-major order)
        for two in range(2):
            nc.sync.dma_start(
                out=t8[0:1, two * KT:(two + 1) * KT],
                in_=temp_d.rearrange("(j two) a b -> (a b) two j", two=2)
                [:, two, :],
            )
        # tcol[p, j] = temperature[2j + p//64]
        for two in range(2):
            nc.sync.dma_start(
                out=tcol[two * DH:(two + 1) * DH, :],
                in_=t8[0:1, two * KT:(two + 1) * KT]
                .unsqueeze(1).broadcast_to((1, DH, KT)),
            )
        for k in range(KT):
            nc.sync.dma_start(out=wpT_sb[:, k, :], in_=wpT_d[k * P:(k + 1) * P, :])
            nc.sync.dma_start(out=wv_sb[:, k, :], in_=wv_d[k * P:(k + 1) * P, :])
            nc.sync.dma_start(out=xTb[:, k, :], in_=xTb_d[k * P:(k + 1) * P, :])

    if "A" in phases_on:
        # --- Xg = x^T x (fp8, DoubleRow over token-tile pairs) ---
        psxg = tc.alloc_tile_pool(name="psxg", bufs=1, space="PSUM")
        xg_ps = [psxg.tile([P, C], F32, tag=f"xg{i}", name=f"xg{i}")
                 for i in range(KT)]
        if CFG.get("xg_dr", True):
            for tp in range(NTP):
                for i in range(KT):
                    nc.tensor.matmul(
                        xg_ps[i][:],
                        x8[:, 2 * tp:2 * tp + 2, i * P:(i + 1) * P],
                        x8[:, 2 * tp:2 * tp + 2, :],
                        start=(tp == 0), stop=(tp == NTP - 1),
                        perf_mode=DR,
                    )
        else:
            for t in range(NT):
                for i in range(KT):
                    nc.tensor.matmul(
                        xg_ps[i][:],
                        x8[:, t, i * P:(i + 1) * P],
                        x8[:, t, :],
                        start=(t == 0), stop=(t == NT - 1),
                    )
        # half-width evacuations on both engines halve the Xg->Z latency
        for i in range(KT):
            nc.vector.tensor_copy(out=xg_sb[:, i, 0:C // 2],
                                  in_=xg_ps[i][:, 0:C // 2])
            nc.scalar.copy(out=xg_sb[:, i, C // 2:C],
                           in_=xg_ps[i][:, C // 2:C])
        psxg.release()

    # fused per-pair-block grams [Q2_p | S_p] (two p-blocks per PSUM bank)
    # + dk row; persists into B
    psq = tc.alloc_tile_pool(name="psq", bufs=1, space="PSUM")
    qs_t = [psq.tile([P, 2, 2, P], F32, tag=f"qs{pp}", name=f"qs{pp}")
            for pp in range(2)]
    # dk broadcast to all partitions directly (all-ones stationary operand)
    dk_ps = psq.tile([P, C], F32, tag="dkps")

    def qs(p):
        return qs_t[p // 2][:, p % 2, :, :]

    if "A" in phases_on:
        # --- Z = Xg @ [wq^T | wk^T]  (bf16), in [cin, qch|kch] layout ---
        # The ||k||^2 ones-matmuls interleave into the Z stream so the
        # rk chain (ln/exp) runs under the QS grams.
        psz = tc.alloc_tile_pool(name="psz", bufs=2, space="PSUM")
        for j in range(KT):
            z_ps = psz.tile([P, 2 * C], F32, tag="z", name=f"z{j}")
            for i in range(KT):
                for half in range(2):
                    nc.tensor.matmul(
                        z_ps[:, half * C:(half + 1) * C],
                        xg_sb[:, i, j * P:(j + 1) * P],
                        wqkT[:, i, half * C:(half + 1) * C],
                        start=(i == 0), stop=(i == KT - 1),
                    )
            nc.vector.tensor_copy(out=z_sb[:, j, 0:C], in_=z_ps[:, 0:C])
            nc.scalar.copy(out=z_sb[:, j, C:2 * C], in_=z_ps[:, C:2 * C])
            # Zk o wk^T product feeding the ||k||^2 ones-matmul
            nc.vector.tensor_mul(
                out=pk_sb[:, j, :], in0=z_sb[:, j, C:2 * C],
                in1=wqkT[:, j, C:2 * C],
            )
            if j >= 1:
                jt = j - 1
                nc.tensor.matmul(dk_ps[:], ones2[:], pk_sb[:, jt, :],
                                 start=(jt == 0), stop=False)
        nc.tensor.matmul(dk_ps[:], ones2[:], pk_sb[:, KT - 1, :],
                         start=False, stop=True)
        psz.release()

        # fused [Q2_p | S_p] grams, group-sequential per p (two p per
        # bank); the diag(Q2) extraction runs on DVE inside the stream
        for p in range(KT):
            for jt in range(KT):
                rhs = wqkT[:, jt, :].rearrange("p (two c) -> p two c", two=2)[
                    :, :, p * P:(p + 1) * P]
                nc.tensor.matmul(
                    qs(p), z_sb[:, jt, p * P:(p + 1) * P], rhs,
                    start=(jt == 0), stop=(jt == KT - 1),
                )
            # diag(Q2_p) -> d2[:, p] inside the gram stream
            nc.vector.tensor_mul(out=scrp[:], in0=qs(p)[:, 0, :], in1=idm[:])
            nc.vector.reduce_sum(d2[:, p:p + 1], scrp[:],
                                 axis=mybir.AxisListType.X)

    if "B" in phases_on:
        with (
            tc.tile_pool(name="smp", bufs=2) as smp,
            tc.tile_pool(name="psw", bufs=2, space="PSUM") as psw,
        ):
            # rk_bcast = rsqrt(dk) on the already-broadcast [P, C] block
            # (ACT runs under the QS grams; no PE/DVE step needed)
            lnk = smp.tile([P, C], F32, tag="lnk")
            nc.scalar.activation(lnk[:], dk_ps[:], mybir.ActivationFunctionType.Ln)
            nc.scalar.activation(rk_bcast[:], lnk[:],
                                 mybir.ActivationFunctionType.Exp,
                                 bias=0.0, scale=-0.5)
            # rq = temp * rsqrt(diag(Q2)); d2 columns already extracted
            lnq = smp.tile([P, KT], F32, tag="lnq")
            nc.scalar.activation(lnq[:], d2[:], mybir.ActivationFunctionType.Ln)
            rsq = smp.tile([P, KT], F32, tag="rsq")
            nc.scalar.activation(rsq[:], lnq[:], mybir.ActivationFunctionType.Exp,
                                 bias=0.0, scale=-0.5)
            nc.vector.tensor_mul(out=rq_col[:], in0=rsq[:], in1=tcol[:])

            # softmax per head pair -> block-diag attn (bf16) -> weff;
            # G blocks 0,1 accumulate inside the weff trickle
            psg2 = tc.alloc_tile_pool(name="psg2", bufs=1, space="PSUM")
            gps = [psg2.tile([P, C], F32, tag=f"gp{j}", name=f"gp{j}")
                   for j in range(2)]
            for p in range(KT):
                # smt = S * rk, + additive block-diag mask; the rq factor
                # rides the exp's per-partition scale (mask is -3000 so it
                # still kills the off-head quadrants after scaling)
                smt = smp.tile([P, P], F32, tag="smt")
                nc.vector.tensor_mul(
                    out=smt[:], in0=qs(p)[:, 1, :],
                    in1=rk_bcast[:, p * P:(p + 1) * P],
                )
                smtm = smp.tile([P, P], F32, tag="smtm")
                nc.vector.tensor_add(out=smtm[:], in0=smt[:], in1=mask_bd[:])
                et = smp.tile([P, P], F32, tag="et")
                ssum = smp.tile([P, 1], F32, tag="ssum")
                rs = smp.tile([P, 1], F32, tag="rs")
                nc.scalar.activation(et[:], smtm[:],
                                     mybir.ActivationFunctionType.Exp,
                                     bias=0.0, scale=rq_col[:, p:p + 1],
                                     accum_out=ssum[:, 0:1])
                nc.vector.reciprocal(rs[:], ssum[:])
                abd = smp.tile([P, P], BF16, tag="abd")
                nc.vector.tensor_scalar_mul(abd[:], et[:], rs[:, 0:1])
                wps = psw.tile([P, C], F32, tag="wps")
                nc.tensor.matmul(wps[:], abd[:], wpT_sb[:, p, :],
                                 start=True, stop=True)
                copy_out(weff[:, p, :], wps[:])
                for j in range(2):
                    nc.tensor.matmul(
                        gps[j][:], wv_sb[:, p, j * P:(j + 1) * P],
                        weff[:, p, :],
                        start=(p == 0), stop=(p == KT - 1),
                    )
            # G blocks 2,3 after the last weff
            for j in range(2):
                copy_out(g_sb[:, j, :], gps[j][:])
            for j in range(2):
                gp = psg2.tile([P, C], F32, tag=f"gp{j}", name=f"gp{j + 2}")
                for t in range(KT):
                    nc.tensor.matmul(
                        gp[:], wv_sb[:, t, (j + 2) * P:(j + 3) * P],
                        weff[:, t, :],
                        start=(t == 0), stop=(t == KT - 1),
                    )
                copy_out(g_sb[:, j + 2, :], gp[:])
            psg2.release()

    stub = []
    if "A" not in phases_on:
        stub = [xg_sb, z_sb, pk_sb, weff, g_sb, rq_col, rk_bcast, d2, rk_row]
    elif "B" not in phases_on:
        stub = [weff, g_sb]
    for t_ in stub:
        nc.gpsimd.memset(t_[:], 0.0)

    psq.release()

    if "C" in phases_on:
        with (
            tc.tile_pool(name="yp", bufs=3) as yp,
            tc.tile_pool(name="psy", bufs=CFG["psy_bufs"], space="PSUM") as psy,
        ):
            # smaller final chunks shorten the last-store tail
            chunks = [4] * 7 + [2] * 2
            g0 = 0
            for npc in chunks:
                yc = yp.tile([P, TPC, C], BF16, tag="yc")
                for t in range(npc):
                    g = g0 + t
                    ps = psy.tile([P, C], F32, tag="psy")
                    for k in range(KT):
                        nc.tensor.matmul(
                            ps[:], xTb[:, k, g * P:(g + 1) * P], g_sb[:, k, :],
                            start=(k == 0), stop=(k == KT - 1),
                        )
                    copy_out(yc[:, t, :], ps[:])
                nc.sync.dma_start(
                    out=out_d[g0 * P:(g0 + npc) * P, :].rearrange(
                        "(t p) c -> p t c", p=P
                    ),
                    in_=yc[:, 0:npc, :],
                )
                g0 += npc
    else:
        for ch in range(NCH):
            nc.sync.dma_start(
                out=out_d[ch * C:(ch + 1) * C, :].rearrange(
                    "(t p) c -> p t c", p=P
                ),
                in_=g_sb[:],
            )


_NC_CACHE = None


def _get_nc():
    global _NC_CACHE
    if _NC_CACHE is None:
        _NC_CACHE = build_bass()
    return _NC_CACHE


def make_in_maps(x, w_qkv, w_proj, temperature):
    """Host-side prep: transpose/cast so the kernel only does plain
    contiguous DMA loads."""
    import ml_dtypes

    bf = ml_dtypes.bfloat16
    f8 = ml_dtypes.float8_e4m3
    x = np.asarray(x, dtype=np.float32)
    w_qkv = np.asarray(w_qkv, dtype=np.float32)
    w_proj = np.asarray(w_proj, dtype=np.float32)
    temperature = np.ascontiguousarray(np.asarray(temperature, dtype=np.float32))

    wq = w_qkv[0:C]          # [512 qch, 512 cin]
    wk = w_qkv[C:2 * C]
    wqkT = np.ascontiguousarray(
        np.concatenate([wq.T, wk.T], axis=1).astype(bf))     # [cin, 1024]
    wv = np.ascontiguousarray(w_qkv[2 * C:3 * C].astype(bf))  # [he, cin]
    wpT = np.ascontiguousarray(w_proj.T.astype(bf))           # [hd, cout]
    ident = np.eye(P, dtype=np.float32)

    maps = []
    for b in range(N_CORES):
        maps.append({
            "x8": np.ascontiguousarray(x[b].astype(f8)),
            "xTb": np.ascontiguousarray(x[b].T.astype(bf)),
            "wqkT": wqkT,
            "wv": wv,
            "wpT": wpT,
            "temperature": temperature,
            "ident": ident,
        })
    return maps


def kernel(**inputs) -> np.ndarray:
    from concourse.bass_utils import run_bass_kernel_spmd

    nc = _get_nc()
    in_maps = make_in_maps(
        inputs["x"], inputs["w_qkv"], inputs["w_proj"], inputs["temperature"]
    )
    res = run_bass_kernel_spmd(nc, in_maps, core_ids=list(range(N_CORES)))
    return np.stack(
        [np.asarray(res.results[b]["out"], dtype=np.float32)
         for b in range(N_CORES)],
        axis=0,
    )
